# revision 12
# baseline (speedup 1.0000x reference)
"""Trainium2 Bass kernel for nn_CWSModel (char-word segmentation model).

Self-contained: takes FULL inputs (as from reference.setup_inputs()), shards
the batch across 8 NeuronCores (4 sequences each, both LSTM directions per
core), runs via bass_utils.run_bass_kernel_spmd, concatenates outputs.

Per-core plan:
  - indirect-DMA gather char/bichar embedding rows, PE-transpose -> xT (bf16)
  - xg = [x; 1] @ [Wih; b].T pre-GEMM for both directions (fp32 PSUM -> bf16)
  - S-step LSTM recurrence per direction. PSUM gate layout:
      pg[p = 32*e + b, f = 200*o + 50*g + j']  (e,o,j') = dest hidden split
      100*e + 50*o + j', gates g ordered (i,f,o,g~). xg rows injected with a
      selector matmul; h-part via one K=100 matmul per (e_src, e_dst).
    sigmoid/tanh on ScalarE, c/h updates on VectorE, PE-transpose h -> hT.
  - FFN is linearized: logits = valid*(Pf[e1]-Pf[k]+Pb[k+1]-Pb[e2])
      + preP[sub] + embP[clip],  Pf/Pb = h @ ffn_w-span-cols.T (tiny on-device
      GEMMs), preP/embP = subword tables pre-projected through ffn_w subword
      cols on the host (weight-only transform; ffn bias folded in halves).
"""
import sys
import numpy as np

if "/opt/trn_rl_repo" not in sys.path:
    sys.path.insert(0, "/opt/trn_rl_repo")

B, W = 32, 8
HD = 200
CHAR_V, BICHAR_V, PRE_V, SUB_V = 10000, 400000, 500000, 100000
UNK = 1
NCORES = 8
BPC = B // NCORES  # 4

_GATE_BASE = np.array([0, 200, 600, 400])  # permuted (i,f,o,g~) -> orig row base


def _bf16(x):
    import ml_dtypes
    return np.ascontiguousarray(np.asarray(x, dtype=np.float32)).astype(ml_dtypes.bfloat16)


def _col_perm():
    """col -> original gate-row index, for col = 400e + 200o + 50g + j'."""
    cols = np.zeros(800, np.int64)
    for gp in range(4):
        for jd in range(HD):
            e, o, jp = jd // 100, (jd % 100) // 50, jd % 50
            cols[400 * e + 200 * o + 50 * gp + jp] = _GATE_BASE[gp] + jd
    return cols


def prep_host(inputs, S):
    f32 = np.float32
    chars = np.asarray(inputs["chars"]).astype(np.int32)
    bichars = np.asarray(inputs["bichars"]).astype(np.int32)
    subwords = np.asarray(inputs["subwords"]).astype(np.int32)
    ffn_w = np.asarray(inputs["ffn_w"], dtype=f32)
    ffn_b = np.asarray(inputs["ffn_b"], dtype=f32)
    cols = _col_perm()

    def mk_wh(Whh):
        return np.asarray(Whh, dtype=f32)[cols, :].T.copy()  # [200, 800]

    def mk_wx(Wih, b):
        Wih = np.asarray(Wih, dtype=f32)
        out = np.zeros((201, 800), f32)
        out[:200] = Wih[cols, :].T
        out[200] = np.asarray(b, dtype=f32)[cols]
        return out

    wh_f, wh_b = mk_wh(inputs["Whh_f"]), mk_wh(inputs["Whh_b"])
    wx_f = mk_wx(inputs["Wih_f"], inputs["b_f"])
    wx_b = mk_wx(inputs["Wih_b"], inputs["b_b"])

    wpf = ffn_w[:, 0:HD].T.copy()          # [200, 4]
    wpb = ffn_w[:, HD:2 * HD].T.copy()
    Ws = ffn_w[:, 2 * HD:2 * HD + 100]
    preP = np.asarray(inputs["pretrained_w"], dtype=f32) @ Ws.T + 0.5 * ffn_b
    embP = np.asarray(inputs["emb_subword_w"], dtype=f32) @ Ws.T + 0.5 * ffn_b

    # replicated at each 32-partition base so lhsT/rhs share a start partition
    isel = np.zeros((128, 8, 32), f32)
    for a in range(4):
        for g in range(8):
            for m in range(4):
                isel[32 * a + 4 * g + m, g, m] = 1.0

    NT = 4 * S // 128
    NQA = NT
    BW = 4 * S * 4 // 128
    # validity mask [128, 8*BW], col within i-block = 16*b + 4*qq + l layout
    # generalized: col = b*(NQA//4)*4 + qq*4 + l, row r = 128*(b*(NQA//4)+qq)+p
    nqb = NQA // 4  # qq tiles per b
    mask = np.zeros((128, 8 * BW), f32)
    for i in range(8):
        for b_ in range(4):
            for qq in range(nqb):
                for p in range(128):
                    k = 128 * qq + p
                    if k + i <= S - 3:
                        c0 = i * BW + (b_ * nqb + qq) * 4
                        mask[p, c0:c0 + 4] = 1.0

    tau = np.arange(4 * S)
    t_of, b_of = tau // 4, tau % 4

    common = {
        "tch": np.ascontiguousarray(np.asarray(inputs["emb_char_w"], dtype=f32)),
        "tbi": np.ascontiguousarray(np.asarray(inputs["emb_bichar_w"], dtype=f32)),
        "preP": np.ascontiguousarray(preP, dtype=f32),
        "embP": np.ascontiguousarray(embP, dtype=f32),
        "wh_f0": _bf16(wh_f[:100]), "wh_f1": _bf16(wh_f[100:]),
        "wh_b0": _bf16(wh_b[:100]), "wh_b1": _bf16(wh_b[100:]),
        "wx_f0": _bf16(np.concatenate([wx_f[0:100], wx_f[200:201]], 0)),
        "wx_f1": _bf16(wx_f[100:200]),
        "wx_b0": _bf16(np.concatenate([wx_b[0:100], wx_b[200:201]], 0)),
        "wx_b1": _bf16(wx_b[100:200]),
        "wpf": _bf16(wpf), "wpb": _bf16(wpb),
        "isel": _bf16(isel.reshape(128, 256)),
        "identf": np.eye(128, dtype=f32),
        "mask": mask,
    }

    in_maps = []
    for c in range(NCORES):
        ch = chars[c * BPC:(c + 1) * BPC, :S]
        bi = bichars[c * BPC:(c + 1) * BPC, :S]
        sw = subwords[c * BPC:(c + 1) * BPC, :S - 2]
        cidx = ch[b_of, t_of].reshape(NT, 128).T.astype(np.int32).copy()
        bidx = bi[b_of, t_of].reshape(NT, 128).T.astype(np.int32).copy()
        r_all = np.arange(128 * NQA)
        b_r, k_r = r_all // S, r_all % S
        ok = k_r < (S - 2)
        spidx = np.zeros((128, 8 * NQA), np.int32)
        scidx = np.zeros((128, 8 * NQA), np.int32)
        for i in range(8):
            v = np.where(ok, sw[b_r, np.minimum(k_r, S - 3), i], 0)
            spidx[:, i * NQA:(i + 1) * NQA] = v.reshape(NQA, 128).T
            scidx[:, i * NQA:(i + 1) * NQA] = np.where(v >= SUB_V, UNK, v).reshape(NQA, 128).T
        m = dict(common)
        m.update({"cidx": cidx, "bidx": bidx, "spidx": spidx, "scidx": scidx})
        in_maps.append(m)
    return in_maps


def build_program(S):
    from contextlib import ExitStack
    from concourse import bass, mybir, bacc
    import concourse.tile as tile

    f32 = mybir.dt.float32
    bf16 = mybir.dt.bfloat16
    i32 = mybir.dt.int32
    AF = mybir.ActivationFunctionType
    OP = mybir.AluOpType

    NT = 4 * S // 128
    NQA = NT
    BW = 4 * S * 4 // 128
    nqb = NQA // 4
    SP = S + 12
    SK = S - 2

    nc = bacc.Bacc("TRN2", target_bir_lowering=False, debug=False,
                   enable_asserts=False, num_devices=NCORES)

    def din(name, shape, dt):
        return nc.dram_tensor(name, list(shape), dt, kind="ExternalInput").ap()

    tch = din("tch", [CHAR_V, 100], f32)
    tbi = din("tbi", [BICHAR_V, 100], f32)
    preP = din("preP", [PRE_V, 4], f32)
    embP = din("embP", [SUB_V, 4], f32)
    wh_d = {(0, 0): din("wh_f0", [100, 800], bf16), (0, 1): din("wh_f1", [100, 800], bf16),
            (1, 0): din("wh_b0", [100, 800], bf16), (1, 1): din("wh_b1", [100, 800], bf16)}
    wx_d = {(0, 0): din("wx_f0", [101, 800], bf16), (0, 1): din("wx_f1", [100, 800], bf16),
            (1, 0): din("wx_b0", [101, 800], bf16), (1, 1): din("wx_b1", [100, 800], bf16)}
    wpf_d = din("wpf", [200, 4], bf16)
    wpb_d = din("wpb", [200, 4], bf16)
    isel_d = din("isel", [128, 256], bf16)
    identf_d = din("identf", [128, 128], f32)
    mask_d = din("mask", [128, 8 * BW], f32)
    cidx_d = din("cidx", [128, NT], i32)
    bidx_d = din("bidx", [128, NT], i32)
    spidx_d = din("spidx", [128, 8 * NQA], i32)
    scidx_d = din("scidx", [128, 8 * NQA], i32)

    out_d = nc.dram_tensor("out", [BPC, SK, 8, 4], f32, kind="ExternalOutput").ap()
    p_dram = [nc.dram_tensor(n, [BPC, SP, 4], f32).ap() for n in ("pf_s", "pb_s")]

    with ExitStack() as ctx:
        tc = ctx.enter_context(tile.TileContext(nc))
        const = ctx.enter_context(tc.tile_pool(name="const", bufs=1))
        big = ctx.enter_context(tc.tile_pool(name="big", bufs=1))
        work = ctx.enter_context(tc.tile_pool(name="work", bufs=3))
        step = ctx.enter_context(tc.tile_pool(name="step", bufs=4))
        psum = ctx.enter_context(tc.tile_pool(name="psum", bufs=6, space="PSUM"))

        def load_const(ap, dt):
            t = const.tile(list(ap.shape), dt, tag=f"c_{ap.tensor.name}", name=f"c_{ap.tensor.name}")
            nc.sync.dma_start(t[:], ap)
            return t

        wh_t = {k: load_const(v, bf16) for k, v in wh_d.items()}
        wx_t = {k: load_const(v, bf16) for k, v in wx_d.items()}
        wp_t = []  # [dir][chunk] -> [100, 4]
        for d, src in enumerate((wpf_d, wpb_d)):
            chunks = []
            for e in (0, 1):
                t = const.tile([100, 4], bf16, tag=f"wp{d}{e}", name=f"wp{d}{e}")
                nc.sync.dma_start(t[:], src[100 * e:100 * e + 100, :])
                chunks.append(t)
            wp_t.append(chunks)
        isel_t = load_const(isel_d, bf16)
        identf_t = load_const(identf_d, f32)
        mask_t = load_const(mask_d, f32)
        cidx_t = load_const(cidx_d, i32)
        bidx_t = load_const(bidx_d, i32)
        spidx_t = load_const(spidx_d, i32)
        scidx_t = load_const(scidx_d, i32)
        zeros_t = const.tile([32, 4], f32, tag="zeros")
        nc.vector.memset(zeros_t[:], 0.0)
        zsel_t = const.tile([128, 32], bf16, tag="zsel")
        nc.vector.memset(zsel_t[:], 0.0)

        # ---------- embedding gathers + transposes -> xT ----------
        xT0 = big.tile([101, 4 * S], bf16, tag="xT0")
        xT1 = big.tile([100, 4 * S], bf16, tag="xT1")
        nc.vector.memset(xT0[:], 1.0)  # row 100 stays as ones-row
        for q in range(NT):
            for tbl, idx, dst in ((tch, cidx_t, xT0), (tbi, bidx_t, xT1)):
                stg = work.tile([128, 100], f32, tag="xstage")
                nc.gpsimd.indirect_dma_start(
                    out=stg[:], out_offset=None, in_=tbl,
                    in_offset=bass.IndirectOffsetOnAxis(ap=idx[:, q:q + 1], axis=0))
                pt = psum.tile([100, 128], f32, tag="ps")
                nc.tensor.transpose(out=pt[:], in_=stg[:], identity=identf_t[:])
                nc.vector.tensor_copy(dst[0:100, q * 128:(q + 1) * 128], pt[:])

        # ---------- xg pre-GEMM ----------
        xg = [big.tile([128, NT * 800], bf16, tag=f"xg{d}", name=f"xg{d}") for d in (0, 1)]
        for d in (0, 1):
            for q in range(NT):
                for half in (0, 1):
                    pg = psum.tile([128, 400], f32, tag="ps")
                    nc.tensor.matmul(out=pg[:], lhsT=xT0[:, q * 128:(q + 1) * 128],
                                     rhs=wx_t[(d, 0)][:, half * 400:half * 400 + 400],
                                     start=True, stop=False)
                    nc.tensor.matmul(out=pg[:], lhsT=xT1[:, q * 128:(q + 1) * 128],
                                     rhs=wx_t[(d, 1)][:, half * 400:half * 400 + 400],
                                     start=False, stop=True)
                    nc.vector.tensor_copy(
                        xg[d][:, q * 800 + half * 400:q * 800 + half * 400 + 400], pg[:])

        # ---------- subword gathers (overlap with recurrence) ----------
        subp_all = big.tile([128, 8 * BW], f32, tag="subp")
        sube_all = big.tile([128, 8 * BW], f32, tag="sube")
        for i in range(8):
            for q in range(NQA):
                col = i * NQA + q
                for tbl, idx, dst in ((preP, spidx_t, subp_all), (embP, scidx_t, sube_all)):
                    nc.gpsimd.indirect_dma_start(
                        out=dst[:, i * BW + q * 4:i * BW + q * 4 + 4],
                        out_offset=None, in_=tbl,
                        in_offset=bass.IndirectOffsetOnAxis(ap=idx[:, col:col + 1], axis=0))

        # ---------- LSTM recurrence ----------
        hT = [big.tile([100, 2 * 4 * SP], bf16, tag=f"hT{d}", name=f"hT{d}") for d in (0, 1)]
        c_st = [big.tile([64, 100], f32, tag=f"c{d}", name=f"c{d}") for d in (0, 1)]
        for d in (0, 1):
            nc.vector.memset(c_st[d][:], 0.0)

        for t in range(S):
            for d in (0, 1):
                tx = t if d == 0 else S - 1 - t
                pg = psum.tile([64, 400], f32, tag="ps")
                q, a, g8 = tx // 32, (tx % 32) // 8, (tx % 32) % 8
                for e in (0, 1):
                    nc.tensor.matmul(
                        out=pg[32 * e:32 * e + 32, :],
                        lhsT=isel_t[32 * a:32 * a + 32, g8 * 32:(g8 + 1) * 32],
                        rhs=xg[d][32 * a:32 * a + 32,
                                  q * 800 + 400 * e:q * 800 + 400 * e + 400],
                        start=True, stop=False,
                        tile_position=(32 * a, 32 * e))
                if t > 0:
                    tp = tx + 1 if d == 1 else tx - 1
                    hview = hT[d][:].rearrange("p (e b t) -> p e b t", e=2, b=4)
                    for e_dst in (0, 1):
                        for e_src in (0, 1):
                            nc.tensor.matmul(
                                out=pg[32 * e_dst:32 * e_dst + 4, :],
                                lhsT=hview[:, e_src, :, tp],
                                rhs=wh_t[(d, e_src)][:, 400 * e_dst:400 * e_dst + 400],
                                start=False, stop=False)
                for e in (0, 1):
                    # zero-adding N=1 matmul: marks the bank region stopped
                    nc.tensor.matmul(
                        out=pg[32 * e:32 * e + 32, 0:1],
                        lhsT=zsel_t[32 * a:32 * a + 32, :],
                        rhs=xg[d][32 * a:32 * a + 32, 0:1],
                        start=False, stop=True,
                        tile_position=(32 * a, 32 * e))
                # ---- gates ----
                sg = step.tile([64, 400], f32, tag="sg")
                pgv = pg[:].rearrange("p (o x) -> p o x", o=2)
                sgv = sg[:].rearrange("p (o x) -> p o x", o=2)
                nc.scalar.activation(sgv[:, :, 0:150], pgv[:, :, 0:150], AF.Sigmoid)
                nc.scalar.activation(sgv[:, :, 150:200], pgv[:, :, 150:200], AF.Tanh)
                g4 = sg[:].rearrange("p (o g j) -> p o g j", o=2, g=4)
                si, sf, so, tg = g4[:, :, 0, :], g4[:, :, 1, :], g4[:, :, 2, :], g4[:, :, 3, :]
                u = step.tile([64, 100], f32, tag="u")
                m1 = step.tile([64, 100], f32, tag="m1")
                uv = u[:].rearrange("p (o j) -> p o j", o=2)
                m1v = m1[:].rearrange("p (o j) -> p o j", o=2)
                cv = c_st[d][:].rearrange("p (o j) -> p o j", o=2)
                nc.vector.tensor_tensor(out=m1v, in0=sf, in1=cv, op=OP.mult)
                nc.vector.tensor_tensor(out=uv, in0=si, in1=tg, op=OP.mult)
                nc.vector.tensor_tensor(out=cv, in0=m1v, in1=uv, op=OP.add)
                tct = step.tile([64, 100], f32, tag="tct")
                nc.scalar.activation(tct[:], c_st[d][:], AF.Tanh)
                h = step.tile([64, 100], f32, tag="h")
                nc.vector.tensor_tensor(
                    out=h[:].rearrange("p (o j) -> p o j", o=2), in0=so,
                    in1=tct[:].rearrange("p (o j) -> p o j", o=2), op=OP.mult)
                # ---- transpose h -> hT cols ----
                pt = psum.tile([100, 64], f32, tag="ps")
                nc.tensor.transpose(out=pt[:], in_=h[:], identity=identf_t[0:64, 0:64])
                src = pt[:].rearrange("p (e r) -> p e r", e=2)[:, :, 0:4]
                dst = hT[d][:].rearrange("p (e b t) -> p e b t", e=2, b=4)[:, :, :, tx]
                nc.vector.tensor_copy(dst, src)

        # ---------- P projections -> DRAM staging ----------
        for d in (0, 1):
            hview = hT[d][:].rearrange("p (e b t) -> p e b t", e=2, b=4)
            for b_ in range(4):
                for m in range(S // 128):
                    pp = psum.tile([128, 4], f32, tag="ps")
                    for e in (0, 1):
                        nc.tensor.matmul(
                            out=pp[:], lhsT=hview[:, e, b_, 128 * m:128 * m + 128],
                            rhs=wp_t[d][e][:], start=(e == 0), stop=(e == 1))
                    pst = work.tile([128, 4], f32, tag="pstage")
                    nc.vector.tensor_copy(pst[:], pp[:])
                    nc.sync.dma_start(p_dram[d][b_, 128 * m:128 * m + 128, :], pst[:])
                nc.sync.dma_start(p_dram[d][b_, S:SP, :], zeros_t[0:SP - S, :])

        # ---------- FFN assembly ----------
        def pload(dram, shift, tag):
            t = work.tile([128, BW], f32, tag=tag, name=tag)
            dstv = t[:].rearrange("p (b qq l) -> p b qq l", b=4, qq=nqb)
            for b_ in range(4):
                src = dram[b_, shift:shift + S, :].rearrange("(qq p) l -> p qq l", p=128)
                nc.sync.dma_start(dstv[:, b_, :, :], src)
            return t

        pfk = pload(p_dram[0], 0, "pfk")
        pbk1 = pload(p_dram[1], 1, "pbk1")
        n_full = SK // 128
        rem = SK % 128
        for i in range(8):
            pfe1 = pload(p_dram[0], i + 1, "pfe1")
            pbe2 = pload(p_dram[1], i + 2, "pbe2")
            d1 = work.tile([128, BW], f32, tag="d1")
            d2 = work.tile([128, BW], f32, tag="d2")
            lg = work.tile([128, BW], f32, tag="lg")
            nc.vector.tensor_tensor(out=d1[:], in0=pfe1[:], in1=pfk[:], op=OP.subtract)
            nc.vector.tensor_tensor(out=d2[:], in0=pbk1[:], in1=pbe2[:], op=OP.subtract)
            nc.vector.tensor_tensor(out=d1[:], in0=d1[:], in1=d2[:], op=OP.add)
            nc.vector.tensor_tensor(out=d1[:], in0=d1[:],
                                    in1=mask_t[:, i * BW:(i + 1) * BW], op=OP.mult)
            nc.vector.tensor_tensor(out=d1[:], in0=d1[:],
                                    in1=subp_all[:, i * BW:(i + 1) * BW], op=OP.add)
            nc.vector.tensor_tensor(out=lg[:], in0=d1[:],
                                    in1=sube_all[:, i * BW:(i + 1) * BW], op=OP.add)
            lgv = lg[:].rearrange("p (b qq l) -> p b qq l", b=4, qq=nqb)
            for b_ in range(4):
                if n_full > 0:
                    dstv = out_d[b_, 0:128 * n_full, i, :].rearrange(
                        "(qq p) l -> p qq l", p=128)
                    nc.sync.dma_start(dstv, lgv[:, b_, 0:n_full, :])
                if rem > 0:
                    dstv = out_d[b_, 128 * n_full:SK, i, :]
                    nc.sync.dma_start(dstv, lgv[0:rem, b_, n_full, :])

    nc.compile()
    return nc


def get_program(S):
    import kernel as _self  # noqa
    key = ("prog", S)
    if key not in _prog_cache:
        _prog_cache[key] = build_program(S)
    return _prog_cache[key]


_prog_cache = {}


def kernel(**inputs):
    S = int(np.asarray(inputs["chars"]).shape[1])
    nc = get_program(S)
    in_maps = prep_host(inputs, S)
    from concourse import bass_utils
    res = bass_utils.run_bass_kernel_spmd(nc, in_maps, list(range(NCORES)))
    outs = [res.results[c]["out"] for c in range(NCORES)]
    return np.concatenate(outs, axis=0).astype(np.float32)


# revision 14
# speedup vs baseline: 1.4321x; 1.4321x over previous
"""Trainium2 Bass kernel for nn_CWSModel (char-word segmentation model).

Self-contained: takes FULL inputs (as from reference.setup_inputs()), shards
the batch across 8 NeuronCores (4 sequences each, both LSTM directions per
core), runs via bass_utils.run_bass_kernel_spmd, concatenates outputs.

Per-core plan:
  - indirect-DMA gather char/bichar embedding rows, PE-transpose -> xT (bf16)
  - xg = [x; 1] @ [Wih; b].T pre-GEMM for both directions (fp32 PSUM -> bf16)
  - S-step LSTM recurrence per direction. PSUM gate layout:
      pg[p = 32*e + b, f = 200*o + 50*g + j']  (e,o,j') = dest hidden split
      100*e + 50*o + j', gates g ordered (i,f,o,g~). xg rows injected with a
      selector matmul; h-part via one K=100 matmul per (e_src, e_dst).
    sigmoid/tanh on ScalarE, c/h updates on VectorE, PE-transpose h -> hT.
  - FFN is linearized: logits = valid*(Pf[e1]-Pf[k]+Pb[k+1]-Pb[e2])
      + preP[sub] + embP[clip],  Pf/Pb = h @ ffn_w-span-cols.T (tiny on-device
      GEMMs), preP/embP = subword tables pre-projected through ffn_w subword
      cols on the host (weight-only transform; ffn bias folded in halves).
"""
import sys
import numpy as np

if "/opt/trn_rl_repo" not in sys.path:
    sys.path.insert(0, "/opt/trn_rl_repo")

B, W = 32, 8
HD = 200
CHAR_V, BICHAR_V, PRE_V, SUB_V = 10000, 400000, 500000, 100000
UNK = 1
NCORES = 8
BPC = B // NCORES  # 4

_GATE_BASE = np.array([0, 200, 600, 400])  # permuted (i,f,o,g~) -> orig row base


def _bf16(x):
    import ml_dtypes
    return np.ascontiguousarray(np.asarray(x, dtype=np.float32)).astype(ml_dtypes.bfloat16)


def _col_perm():
    """col -> original gate-row index, for col = 400e + 200o + 50g + j'."""
    cols = np.zeros(800, np.int64)
    for gp in range(4):
        for jd in range(HD):
            e, o, jp = jd // 100, (jd % 100) // 50, jd % 50
            cols[400 * e + 200 * o + 50 * gp + jp] = _GATE_BASE[gp] + jd
    return cols


def prep_host(inputs, S):
    f32 = np.float32
    chars = np.asarray(inputs["chars"]).astype(np.int32)
    bichars = np.asarray(inputs["bichars"]).astype(np.int32)
    subwords = np.asarray(inputs["subwords"]).astype(np.int32)
    ffn_w = np.asarray(inputs["ffn_w"], dtype=f32)
    ffn_b = np.asarray(inputs["ffn_b"], dtype=f32)
    cols = _col_perm()

    def mk_wh(Whh):
        return np.asarray(Whh, dtype=f32)[cols, :].T.copy()  # [200, 800]

    def mk_wx(Wih, b):
        Wih = np.asarray(Wih, dtype=f32)
        out = np.zeros((201, 800), f32)
        out[:200] = Wih[cols, :].T
        out[200] = np.asarray(b, dtype=f32)[cols]
        return out

    wh_f, wh_b = mk_wh(inputs["Whh_f"]), mk_wh(inputs["Whh_b"])
    wx_f = mk_wx(inputs["Wih_f"], inputs["b_f"])
    wx_b = mk_wx(inputs["Wih_b"], inputs["b_b"])

    wpf = ffn_w[:, 0:HD].T.copy()          # [200, 4]
    wpb = ffn_w[:, HD:2 * HD].T.copy()
    Ws = ffn_w[:, 2 * HD:2 * HD + 100]
    preP = np.asarray(inputs["pretrained_w"], dtype=f32) @ Ws.T + 0.5 * ffn_b
    embP = np.asarray(inputs["emb_subword_w"], dtype=f32) @ Ws.T + 0.5 * ffn_b

    # replicated at each 32-partition base so lhsT/rhs share a start partition
    isel = np.zeros((128, 8, 32), f32)
    for a in range(4):
        for g in range(8):
            for m in range(4):
                isel[32 * a + 4 * g + m, g, m] = 1.0

    NT = 4 * S // 128
    NQA = NT
    BW = 4 * S * 4 // 128
    # validity mask [128, 8*BW], col within i-block = 16*b + 4*qq + l layout
    # generalized: col = b*(NQA//4)*4 + qq*4 + l, row r = 128*(b*(NQA//4)+qq)+p
    nqb = NQA // 4  # qq tiles per b
    mask = np.zeros((128, 8 * BW), f32)
    for i in range(8):
        for b_ in range(4):
            for qq in range(nqb):
                for p in range(128):
                    k = 128 * qq + p
                    if k + i <= S - 3:
                        c0 = i * BW + (b_ * nqb + qq) * 4
                        mask[p, c0:c0 + 4] = 1.0

    tau = np.arange(4 * S)
    t_of, b_of = tau // 4, tau % 4

    common = {
        "tch": np.ascontiguousarray(np.asarray(inputs["emb_char_w"], dtype=f32)),
        "tbi": np.ascontiguousarray(np.asarray(inputs["emb_bichar_w"], dtype=f32)),
        "preP": np.ascontiguousarray(preP, dtype=f32),
        "embP": np.ascontiguousarray(embP, dtype=f32),
        "wh_f0": _bf16(wh_f[:100]), "wh_f1": _bf16(wh_f[100:]),
        "wh_b0": _bf16(wh_b[:100]), "wh_b1": _bf16(wh_b[100:]),
        "wx_f0": _bf16(np.concatenate([wx_f[0:100], wx_f[200:201]], 0)),
        "wx_f1": _bf16(wx_f[100:200]),
        "wx_b0": _bf16(np.concatenate([wx_b[0:100], wx_b[200:201]], 0)),
        "wx_b1": _bf16(wx_b[100:200]),
        "wpf": _bf16(wpf), "wpb": _bf16(wpb),
        "isel": _bf16(isel.reshape(128, 256)),
        "identf": np.eye(128, dtype=f32),
        "identb": _bf16(np.eye(128)),
        "mask": mask,
    }

    in_maps = []
    for c in range(NCORES):
        ch = chars[c * BPC:(c + 1) * BPC, :S]
        bi = bichars[c * BPC:(c + 1) * BPC, :S]
        sw = subwords[c * BPC:(c + 1) * BPC, :S - 2]
        cidx = ch[b_of, t_of].reshape(NT, 128).T.astype(np.int32).copy()
        bidx = bi[b_of, t_of].reshape(NT, 128).T.astype(np.int32).copy()
        r_all = np.arange(128 * NQA)
        b_r, k_r = r_all // S, r_all % S
        ok = k_r < (S - 2)
        spidx = np.zeros((128, 8 * NQA), np.int32)
        scidx = np.zeros((128, 8 * NQA), np.int32)
        for i in range(8):
            v = np.where(ok, sw[b_r, np.minimum(k_r, S - 3), i], 0)
            spidx[:, i * NQA:(i + 1) * NQA] = v.reshape(NQA, 128).T
            scidx[:, i * NQA:(i + 1) * NQA] = np.where(v >= SUB_V, UNK, v).reshape(NQA, 128).T
        m = dict(common)
        m.update({"cidx": cidx, "bidx": bidx, "spidx": spidx, "scidx": scidx})
        in_maps.append(m)
    return in_maps


def build_program(S):
    from contextlib import ExitStack
    from concourse import bass, mybir, bacc
    import concourse.tile as tile

    f32 = mybir.dt.float32
    bf16 = mybir.dt.bfloat16
    i32 = mybir.dt.int32
    AF = mybir.ActivationFunctionType
    OP = mybir.AluOpType

    NT = 4 * S // 128
    NQA = NT
    BW = 4 * S * 4 // 128
    nqb = NQA // 4
    SP = S + 12
    SK = S - 2

    nc = bacc.Bacc("TRN2", target_bir_lowering=False, debug=False,
                   enable_asserts=False, num_devices=NCORES)

    def din(name, shape, dt):
        return nc.dram_tensor(name, list(shape), dt, kind="ExternalInput").ap()

    tch = din("tch", [CHAR_V, 100], f32)
    tbi = din("tbi", [BICHAR_V, 100], f32)
    preP = din("preP", [PRE_V, 4], f32)
    embP = din("embP", [SUB_V, 4], f32)
    wh_d = {(0, 0): din("wh_f0", [100, 800], bf16), (0, 1): din("wh_f1", [100, 800], bf16),
            (1, 0): din("wh_b0", [100, 800], bf16), (1, 1): din("wh_b1", [100, 800], bf16)}
    wx_d = {(0, 0): din("wx_f0", [101, 800], bf16), (0, 1): din("wx_f1", [100, 800], bf16),
            (1, 0): din("wx_b0", [101, 800], bf16), (1, 1): din("wx_b1", [100, 800], bf16)}
    wpf_d = din("wpf", [200, 4], bf16)
    wpb_d = din("wpb", [200, 4], bf16)
    isel_d = din("isel", [128, 256], bf16)
    identf_d = din("identf", [128, 128], f32)
    identb_d = din("identb", [128, 128], bf16)
    mask_d = din("mask", [128, 8 * BW], f32)
    cidx_d = din("cidx", [128, NT], i32)
    bidx_d = din("bidx", [128, NT], i32)
    spidx_d = din("spidx", [128, 8 * NQA], i32)
    scidx_d = din("scidx", [128, 8 * NQA], i32)

    out_d = nc.dram_tensor("out", [BPC, SK, 8, 4], f32, kind="ExternalOutput").ap()
    p_dram = [nc.dram_tensor(n, [BPC, SP, 4], f32).ap() for n in ("pf_s", "pb_s")]

    with ExitStack() as ctx:
        tc = ctx.enter_context(tile.TileContext(nc))
        const = ctx.enter_context(tc.tile_pool(name="const", bufs=1))
        big = ctx.enter_context(tc.tile_pool(name="big", bufs=1))
        work = ctx.enter_context(tc.tile_pool(name="work", bufs=3))
        step = ctx.enter_context(tc.tile_pool(name="step", bufs=4))
        psum = ctx.enter_context(tc.tile_pool(name="psum", bufs=6, space="PSUM"))

        def load_const(ap, dt):
            t = const.tile(list(ap.shape), dt, tag=f"c_{ap.tensor.name}", name=f"c_{ap.tensor.name}")
            nc.sync.dma_start(t[:], ap)
            return t

        wh_t = {k: load_const(v, bf16) for k, v in wh_d.items()}
        wx_t = {k: load_const(v, bf16) for k, v in wx_d.items()}
        wp_t = []  # [dir][chunk] -> [100, 4]
        for d, src in enumerate((wpf_d, wpb_d)):
            chunks = []
            for e in (0, 1):
                t = const.tile([100, 4], bf16, tag=f"wp{d}{e}", name=f"wp{d}{e}")
                nc.sync.dma_start(t[:], src[100 * e:100 * e + 100, :])
                chunks.append(t)
            wp_t.append(chunks)
        isel_t = load_const(isel_d, bf16)
        identf_t = load_const(identf_d, f32)
        identb_t = load_const(identb_d, bf16)
        mask_t = load_const(mask_d, f32)
        cidx_t = load_const(cidx_d, i32)
        bidx_t = load_const(bidx_d, i32)
        spidx_t = load_const(spidx_d, i32)
        scidx_t = load_const(scidx_d, i32)
        zeros_t = const.tile([32, 4], f32, tag="zeros")
        nc.vector.memset(zeros_t[:], 0.0)
        zsel_t = const.tile([128, 32], bf16, tag="zsel")
        nc.vector.memset(zsel_t[:], 0.0)

        # ---------- embedding gathers + transposes -> xT ----------
        xT0 = big.tile([101, 4 * S], bf16, tag="xT0")
        xT1 = big.tile([100, 4 * S], bf16, tag="xT1")
        nc.vector.memset(xT0[:], 1.0)  # row 100 stays as ones-row
        for q in range(NT):
            for tbl, idx, dst in ((tch, cidx_t, xT0), (tbi, bidx_t, xT1)):
                stg = work.tile([128, 100], f32, tag="xstage")
                nc.gpsimd.indirect_dma_start(
                    out=stg[:], out_offset=None, in_=tbl,
                    in_offset=bass.IndirectOffsetOnAxis(ap=idx[:, q:q + 1], axis=0))
                pt = psum.tile([100, 128], f32, tag="ps")
                nc.tensor.transpose(out=pt[:], in_=stg[:], identity=identf_t[:])
                nc.vector.tensor_copy(dst[0:100, q * 128:(q + 1) * 128], pt[:])

        # ---------- xg pre-GEMM ----------
        xg = [big.tile([128, NT * 800], bf16, tag=f"xg{d}", name=f"xg{d}") for d in (0, 1)]
        for d in (0, 1):
            for q in range(NT):
                for half in (0, 1):
                    pg = psum.tile([128, 400], f32, tag="ps")
                    nc.tensor.matmul(out=pg[:], lhsT=xT0[:, q * 128:(q + 1) * 128],
                                     rhs=wx_t[(d, 0)][:, half * 400:half * 400 + 400],
                                     start=True, stop=False)
                    nc.tensor.matmul(out=pg[:], lhsT=xT1[:, q * 128:(q + 1) * 128],
                                     rhs=wx_t[(d, 1)][:, half * 400:half * 400 + 400],
                                     start=False, stop=True)
                    nc.vector.tensor_copy(
                        xg[d][:, q * 800 + half * 400:q * 800 + half * 400 + 400], pg[:])

        # ---------- subword gathers (overlap with recurrence) ----------
        subp_all = big.tile([128, 8 * BW], f32, tag="subp")
        sube_all = big.tile([128, 8 * BW], f32, tag="sube")
        for i in range(8):
            for q in range(NQA):
                col = i * NQA + q
                for tbl, idx, dst in ((preP, spidx_t, subp_all), (embP, scidx_t, sube_all)):
                    nc.gpsimd.indirect_dma_start(
                        out=dst[:, i * BW + q * 4:i * BW + q * 4 + 4],
                        out_offset=None, in_=tbl,
                        in_offset=bass.IndirectOffsetOnAxis(ap=idx[:, col:col + 1], axis=0))

        # ---------- LSTM recurrence ----------
        hT = [big.tile([100, 2 * 4 * SP], bf16, tag=f"hT{d}", name=f"hT{d}") for d in (0, 1)]
        c_st = [big.tile([64, 100], f32, tag=f"c{d}", name=f"c{d}") for d in (0, 1)]
        for d in (0, 1):
            nc.vector.memset(c_st[d][:], 0.0)

        def emit_transpose(d, tx, h):
            # h [64, 100] bf16 -> hT[d] columns at time tx
            pt = psum.tile([100, 64], bf16, tag="ps", name="pt")
            nc.tensor.transpose(out=pt[:], in_=h[:], identity=identb_t[0:64, 0:64])
            src = pt[:].rearrange("p (e r) -> p e r", e=2)[:, :, 0:4]
            dst = hT[d][:].rearrange("p (e b t) -> p e b t", e=2, b=4)[:, :, :, tx]
            nc.vector.tensor_copy(dst, src)

        h_prev = [None, None]
        for t in range(S):
            # ---- A) transpose previous step's h (both dirs) ----
            if t > 0:
                for d in (0, 1):
                    emit_transpose(d, t - 1 if d == 0 else S - t, h_prev[d])
            # ---- B) PE blocks for both dirs ----
            pg_d = []
            for d in (0, 1):
                tx = t if d == 0 else S - 1 - t
                pg = psum.tile([64, 400], f32, tag="ps", name="pg")
                pg_d.append(pg)
                q, a, g8 = tx // 32, (tx % 32) // 8, (tx % 32) % 8
                for e in (0, 1):
                    nc.tensor.matmul(
                        out=pg[32 * e:32 * e + 32, :],
                        lhsT=isel_t[32 * a:32 * a + 32, g8 * 32:(g8 + 1) * 32],
                        rhs=xg[d][32 * a:32 * a + 32,
                                  q * 800 + 400 * e:q * 800 + 400 * e + 400],
                        start=True, stop=False,
                        tile_position=(32 * a, 32 * e))
                if t > 0:
                    tp = tx + 1 if d == 1 else tx - 1
                    hview = hT[d][:].rearrange("p (e b t) -> p e b t", e=2, b=4)
                    for e_dst in (0, 1):
                        for e_src in (0, 1):
                            nc.tensor.matmul(
                                out=pg[32 * e_dst:32 * e_dst + 4, :],
                                lhsT=hview[:, e_src, :, tp],
                                rhs=wh_t[(d, e_src)][:, 400 * e_dst:400 * e_dst + 400],
                                start=False, stop=False)
                for e in (0, 1):
                    # zero-adding N=1 matmul: marks the bank region stopped
                    nc.tensor.matmul(
                        out=pg[32 * e:32 * e + 32, 0:1],
                        lhsT=zsel_t[32 * a:32 * a + 32, :],
                        rhs=xg[d][32 * a:32 * a + 32, 0:1],
                        start=False, stop=True,
                        tile_position=(32 * a, 32 * e))
            # ---- C) gate activations (sigmoids first, both dirs) ----
            sg_d, tct_d = [], []
            for d in (0, 1):
                sg = step.tile([64, 400], f32, tag="sg", name="sg")
                sg_d.append(sg)
                pgv = pg_d[d][:].rearrange("p (o x) -> p o x", o=2)
                sgv = sg[:].rearrange("p (o x) -> p o x", o=2)
                nc.scalar.activation(sgv[:, :, 0:150], pgv[:, :, 0:150], AF.Sigmoid)
                nc.scalar.activation(sgv[:, :, 150:200], pgv[:, :, 150:200], AF.Tanh)
            # ---- D) DVE c-updates (both dirs), then tanh(c), then h ----
            for d in (0, 1):
                sg = sg_d[d]
                g4 = sg[:].rearrange("p (o g j) -> p o g j", o=2, g=4)
                si, sf, tg = g4[:, :, 0, :], g4[:, :, 1, :], g4[:, :, 3, :]
                u = step.tile([64, 100], f32, tag="u", name="u")
                m1 = step.tile([64, 100], f32, tag="m1", name="m1")
                uv = u[:].rearrange("p (o j) -> p o j", o=2)
                m1v = m1[:].rearrange("p (o j) -> p o j", o=2)
                cv = c_st[d][:].rearrange("p (o j) -> p o j", o=2)
                nc.vector.tensor_tensor(out=m1v, in0=sf, in1=cv, op=OP.mult)
                nc.vector.tensor_tensor(out=uv, in0=si, in1=tg, op=OP.mult)
                nc.vector.tensor_tensor(out=cv, in0=m1v, in1=uv, op=OP.add)
            for d in (0, 1):
                tct = step.tile([64, 100], f32, tag="tct", name="tct")
                tct_d.append(tct)
                nc.scalar.activation(tct[:], c_st[d][:], AF.Tanh)
            for d in (0, 1):
                g4 = sg_d[d][:].rearrange("p (o g j) -> p o g j", o=2, g=4)
                so = g4[:, :, 2, :]
                h = step.tile([64, 100], bf16, tag="h", name="h")
                nc.vector.tensor_tensor(
                    out=h[:].rearrange("p (o j) -> p o j", o=2), in0=so,
                    in1=tct_d[d][:].rearrange("p (o j) -> p o j", o=2), op=OP.mult)
                h_prev[d] = h
        for d in (0, 1):
            emit_transpose(d, S - 1 if d == 0 else 0, h_prev[d])

        # ---------- P projections -> DRAM staging ----------
        for d in (0, 1):
            hview = hT[d][:].rearrange("p (e b t) -> p e b t", e=2, b=4)
            for b_ in range(4):
                for m in range(S // 128):
                    pp = psum.tile([128, 4], f32, tag="ps")
                    for e in (0, 1):
                        nc.tensor.matmul(
                            out=pp[:], lhsT=hview[:, e, b_, 128 * m:128 * m + 128],
                            rhs=wp_t[d][e][:], start=(e == 0), stop=(e == 1))
                    pst = work.tile([128, 4], f32, tag="pstage")
                    nc.vector.tensor_copy(pst[:], pp[:])
                    nc.sync.dma_start(p_dram[d][b_, 128 * m:128 * m + 128, :], pst[:])
                nc.sync.dma_start(p_dram[d][b_, S:SP, :], zeros_t[0:SP - S, :])

        # ---------- FFN assembly ----------
        def pload(dram, shift, tag):
            t = work.tile([128, BW], f32, tag=tag, name=tag)
            dstv = t[:].rearrange("p (b qq l) -> p b qq l", b=4, qq=nqb)
            for b_ in range(4):
                src = dram[b_, shift:shift + S, :].rearrange("(qq p) l -> p qq l", p=128)
                nc.sync.dma_start(dstv[:, b_, :, :], src)
            return t

        pfk = pload(p_dram[0], 0, "pfk")
        pbk1 = pload(p_dram[1], 1, "pbk1")
        n_full = SK // 128
        rem = SK % 128
        for i in range(8):
            pfe1 = pload(p_dram[0], i + 1, "pfe1")
            pbe2 = pload(p_dram[1], i + 2, "pbe2")
            d1 = work.tile([128, BW], f32, tag="d1")
            d2 = work.tile([128, BW], f32, tag="d2")
            lg = work.tile([128, BW], f32, tag="lg")
            nc.vector.tensor_tensor(out=d1[:], in0=pfe1[:], in1=pfk[:], op=OP.subtract)
            nc.vector.tensor_tensor(out=d2[:], in0=pbk1[:], in1=pbe2[:], op=OP.subtract)
            nc.vector.tensor_tensor(out=d1[:], in0=d1[:], in1=d2[:], op=OP.add)
            nc.vector.tensor_tensor(out=d1[:], in0=d1[:],
                                    in1=mask_t[:, i * BW:(i + 1) * BW], op=OP.mult)
            nc.vector.tensor_tensor(out=d1[:], in0=d1[:],
                                    in1=subp_all[:, i * BW:(i + 1) * BW], op=OP.add)
            nc.vector.tensor_tensor(out=lg[:], in0=d1[:],
                                    in1=sube_all[:, i * BW:(i + 1) * BW], op=OP.add)
            lgv = lg[:].rearrange("p (b qq l) -> p b qq l", b=4, qq=nqb)
            for b_ in range(4):
                if n_full > 0:
                    dstv = out_d[b_, 0:128 * n_full, i, :].rearrange(
                        "(qq p) l -> p qq l", p=128)
                    nc.sync.dma_start(dstv, lgv[:, b_, 0:n_full, :])
                if rem > 0:
                    dstv = out_d[b_, 128 * n_full:SK, i, :]
                    nc.sync.dma_start(dstv, lgv[0:rem, b_, n_full, :])

    nc.compile()
    return nc


def get_program(S):
    import kernel as _self  # noqa
    key = ("prog", S)
    if key not in _prog_cache:
        _prog_cache[key] = build_program(S)
    return _prog_cache[key]


_prog_cache = {}


def kernel(**inputs):
    S = int(np.asarray(inputs["chars"]).shape[1])
    nc = get_program(S)
    in_maps = prep_host(inputs, S)
    from concourse import bass_utils
    res = bass_utils.run_bass_kernel_spmd(nc, in_maps, list(range(NCORES)))
    outs = [res.results[c]["out"] for c in range(NCORES)]
    return np.concatenate(outs, axis=0).astype(np.float32)


# revision 16
# speedup vs baseline: 1.7484x; 1.2209x over previous
"""Trainium2 Bass kernel for nn_CWSModel (char-word segmentation model).

Self-contained: takes FULL inputs (as from reference.setup_inputs()), shards
the batch across 8 NeuronCores (4 sequences each, both LSTM directions per
core), runs via bass_utils.run_bass_kernel_spmd, concatenates outputs.

Per-core plan:
  - indirect-DMA gather char/bichar embedding rows, PE-transpose -> xT (bf16)
  - xg = [x; 1] @ [Wih; b].T pre-GEMM for both directions (fp32 PSUM -> bf16)
  - S-step LSTM recurrence per direction. PSUM gate layout:
      pg[p = 32*e + b, f = 200*o + 50*g + j']  (e,o,j') = dest hidden split
      100*e + 50*o + j', gates g ordered (i,f,o,g~). xg rows injected with a
      selector matmul; h-part via one K=100 matmul per (e_src, e_dst).
    sigmoid/tanh on ScalarE, c/h updates on VectorE, PE-transpose h -> hT.
  - FFN is linearized: logits = valid*(Pf[e1]-Pf[k]+Pb[k+1]-Pb[e2])
      + preP[sub] + embP[clip],  Pf/Pb = h @ ffn_w-span-cols.T (tiny on-device
      GEMMs), preP/embP = subword tables pre-projected through ffn_w subword
      cols on the host (weight-only transform; ffn bias folded in halves).
"""
import sys
import numpy as np

if "/opt/trn_rl_repo" not in sys.path:
    sys.path.insert(0, "/opt/trn_rl_repo")

B, W = 32, 8
HD = 200
CHAR_V, BICHAR_V, PRE_V, SUB_V = 10000, 400000, 500000, 100000
UNK = 1
NCORES = 8
BPC = B // NCORES  # 4

_GATE_BASE = np.array([0, 200, 600, 400])  # permuted (i,f,o,g~) -> orig row base


def _bf16(x):
    import ml_dtypes
    return np.ascontiguousarray(np.asarray(x, dtype=np.float32)).astype(ml_dtypes.bfloat16)


def _col_perm():
    """col -> original gate-row index, for col = 400e + 100g + 50o + j'."""
    cols = np.zeros(800, np.int64)
    for gp in range(4):
        for jd in range(HD):
            e, r = jd // 100, jd % 100  # r = 50*o + j'
            cols[400 * e + 100 * gp + r] = _GATE_BASE[gp] + jd
    return cols


def prep_host(inputs, S):
    f32 = np.float32
    chars = np.asarray(inputs["chars"]).astype(np.int32)
    bichars = np.asarray(inputs["bichars"]).astype(np.int32)
    subwords = np.asarray(inputs["subwords"]).astype(np.int32)
    ffn_w = np.asarray(inputs["ffn_w"], dtype=f32)
    ffn_b = np.asarray(inputs["ffn_b"], dtype=f32)
    cols = _col_perm()

    def mk_wh(Whh):
        return np.asarray(Whh, dtype=f32)[cols, :].T.copy()  # [200, 800]

    def mk_wx(Wih, b):
        Wih = np.asarray(Wih, dtype=f32)
        out = np.zeros((201, 800), f32)
        out[:200] = Wih[cols, :].T
        out[200] = np.asarray(b, dtype=f32)[cols]
        return out

    wh_f, wh_b = mk_wh(inputs["Whh_f"]), mk_wh(inputs["Whh_b"])
    wx_f = mk_wx(inputs["Wih_f"], inputs["b_f"])
    wx_b = mk_wx(inputs["Wih_b"], inputs["b_b"])

    wpf = ffn_w[:, 0:HD].T.copy()          # [200, 4]
    wpb = ffn_w[:, HD:2 * HD].T.copy()
    Ws = ffn_w[:, 2 * HD:2 * HD + 100]
    preP = np.asarray(inputs["pretrained_w"], dtype=f32) @ Ws.T + 0.5 * ffn_b
    embP = np.asarray(inputs["emb_subword_w"], dtype=f32) @ Ws.T + 0.5 * ffn_b

    # replicated at each 32-partition base so lhsT/rhs share a start partition
    isel = np.zeros((128, 8, 32), f32)
    for a in range(4):
        for g in range(8):
            for m in range(4):
                isel[32 * a + 4 * g + m, g, m] = 1.0

    NT = 4 * S // 128
    NQA = NT
    BW = 4 * S * 4 // 128
    # validity mask [128, 8*BW], col within i-block = 16*b + 4*qq + l layout
    # generalized: col = b*(NQA//4)*4 + qq*4 + l, row r = 128*(b*(NQA//4)+qq)+p
    nqb = NQA // 4  # qq tiles per b
    mask = np.zeros((128, 8 * BW), f32)
    for i in range(8):
        for b_ in range(4):
            for qq in range(nqb):
                for p in range(128):
                    k = 128 * qq + p
                    if k + i <= S - 3:
                        c0 = i * BW + (b_ * nqb + qq) * 4
                        mask[p, c0:c0 + 4] = 1.0

    tau = np.arange(4 * S)
    t_of, b_of = tau // 4, tau % 4

    common = {
        "tch": np.ascontiguousarray(np.asarray(inputs["emb_char_w"], dtype=f32)),
        "tbi": np.ascontiguousarray(np.asarray(inputs["emb_bichar_w"], dtype=f32)),
        "preP": np.ascontiguousarray(preP, dtype=f32),
        "embP": np.ascontiguousarray(embP, dtype=f32),
        "wh_f0": _bf16(wh_f[:100]), "wh_f1": _bf16(wh_f[100:]),
        "wh_b0": _bf16(wh_b[:100]), "wh_b1": _bf16(wh_b[100:]),
        "wx_f0": _bf16(np.concatenate([wx_f[0:100], wx_f[200:201]], 0)),
        "wx_f1": _bf16(wx_f[100:200]),
        "wx_b0": _bf16(np.concatenate([wx_b[0:100], wx_b[200:201]], 0)),
        "wx_b1": _bf16(wx_b[100:200]),
        "wpf": _bf16(wpf), "wpb": _bf16(wpb),
        "isel": _bf16(isel.reshape(128, 256)),
        "identf": np.eye(128, dtype=f32),
        "identb": _bf16(np.eye(128)),
        "mask": mask,
    }

    in_maps = []
    for c in range(NCORES):
        ch = chars[c * BPC:(c + 1) * BPC, :S]
        bi = bichars[c * BPC:(c + 1) * BPC, :S]
        sw = subwords[c * BPC:(c + 1) * BPC, :S - 2]
        cidx = ch[b_of, t_of].reshape(NT, 128).T.astype(np.int32).copy()
        bidx = bi[b_of, t_of].reshape(NT, 128).T.astype(np.int32).copy()
        r_all = np.arange(128 * NQA)
        b_r, k_r = r_all // S, r_all % S
        ok = k_r < (S - 2)
        spidx = np.zeros((128, 8 * NQA), np.int32)
        scidx = np.zeros((128, 8 * NQA), np.int32)
        for i in range(8):
            v = np.where(ok, sw[b_r, np.minimum(k_r, S - 3), i], 0)
            spidx[:, i * NQA:(i + 1) * NQA] = v.reshape(NQA, 128).T
            scidx[:, i * NQA:(i + 1) * NQA] = np.where(v >= SUB_V, UNK, v).reshape(NQA, 128).T
        m = dict(common)
        m.update({"cidx": cidx, "bidx": bidx, "spidx": spidx, "scidx": scidx})
        in_maps.append(m)
    return in_maps


def build_program(S):
    from contextlib import ExitStack
    from concourse import bass, mybir, bacc
    import concourse.tile as tile

    f32 = mybir.dt.float32
    bf16 = mybir.dt.bfloat16
    i32 = mybir.dt.int32
    AF = mybir.ActivationFunctionType
    OP = mybir.AluOpType

    NT = 4 * S // 128
    NQA = NT
    BW = 4 * S * 4 // 128
    nqb = NQA // 4
    SP = S + 12
    SK = S - 2

    nc = bacc.Bacc("TRN2", target_bir_lowering=False, debug=False,
                   enable_asserts=False, num_devices=NCORES)

    def din(name, shape, dt):
        return nc.dram_tensor(name, list(shape), dt, kind="ExternalInput").ap()

    tch = din("tch", [CHAR_V, 100], f32)
    tbi = din("tbi", [BICHAR_V, 100], f32)
    preP = din("preP", [PRE_V, 4], f32)
    embP = din("embP", [SUB_V, 4], f32)
    wh_d = {(0, 0): din("wh_f0", [100, 800], bf16), (0, 1): din("wh_f1", [100, 800], bf16),
            (1, 0): din("wh_b0", [100, 800], bf16), (1, 1): din("wh_b1", [100, 800], bf16)}
    wx_d = {(0, 0): din("wx_f0", [101, 800], bf16), (0, 1): din("wx_f1", [100, 800], bf16),
            (1, 0): din("wx_b0", [101, 800], bf16), (1, 1): din("wx_b1", [100, 800], bf16)}
    wpf_d = din("wpf", [200, 4], bf16)
    wpb_d = din("wpb", [200, 4], bf16)
    isel_d = din("isel", [128, 256], bf16)
    identf_d = din("identf", [128, 128], f32)
    identb_d = din("identb", [128, 128], bf16)
    mask_d = din("mask", [128, 8 * BW], f32)
    cidx_d = din("cidx", [128, NT], i32)
    bidx_d = din("bidx", [128, NT], i32)
    spidx_d = din("spidx", [128, 8 * NQA], i32)
    scidx_d = din("scidx", [128, 8 * NQA], i32)

    out_d = nc.dram_tensor("out", [BPC, SK, 8, 4], f32, kind="ExternalOutput").ap()
    p_dram = [nc.dram_tensor(n, [BPC, SP, 4], f32).ap() for n in ("pf_s", "pb_s")]

    with ExitStack() as ctx:
        tc = ctx.enter_context(tile.TileContext(nc))
        const = ctx.enter_context(tc.tile_pool(name="const", bufs=1))
        big = ctx.enter_context(tc.tile_pool(name="big", bufs=1))
        work = ctx.enter_context(tc.tile_pool(name="work", bufs=3))
        step = ctx.enter_context(tc.tile_pool(name="step", bufs=4))
        psum = ctx.enter_context(tc.tile_pool(name="psum", bufs=8, space="PSUM"))

        def load_const(ap, dt):
            t = const.tile(list(ap.shape), dt, tag=f"c_{ap.tensor.name}", name=f"c_{ap.tensor.name}")
            nc.sync.dma_start(t[:], ap)
            return t

        wh_t = {k: load_const(v, bf16) for k, v in wh_d.items()}
        wx_t = {k: load_const(v, bf16) for k, v in wx_d.items()}
        wp_t = []  # [dir][chunk] -> [100, 4]
        for d, src in enumerate((wpf_d, wpb_d)):
            chunks = []
            for e in (0, 1):
                t = const.tile([100, 4], bf16, tag=f"wp{d}{e}", name=f"wp{d}{e}")
                nc.sync.dma_start(t[:], src[100 * e:100 * e + 100, :])
                chunks.append(t)
            wp_t.append(chunks)
        isel_t = load_const(isel_d, bf16)
        identf_t = load_const(identf_d, f32)
        identb_t = load_const(identb_d, bf16)
        mask_t = load_const(mask_d, f32)
        cidx_t = load_const(cidx_d, i32)
        bidx_t = load_const(bidx_d, i32)
        spidx_t = load_const(spidx_d, i32)
        scidx_t = load_const(scidx_d, i32)
        zeros_t = const.tile([32, 4], f32, tag="zeros")
        nc.vector.memset(zeros_t[:], 0.0)
        zsel_t = const.tile([128, 32], bf16, tag="zsel")
        nc.vector.memset(zsel_t[:], 0.0)

        # ---------- embedding gathers + transposes -> xT ----------
        xT0 = big.tile([101, 4 * S], bf16, tag="xT0")
        xT1 = big.tile([100, 4 * S], bf16, tag="xT1")
        nc.vector.memset(xT0[:], 1.0)  # row 100 stays as ones-row
        for q in range(NT):
            for tbl, idx, dst in ((tch, cidx_t, xT0), (tbi, bidx_t, xT1)):
                stg = work.tile([128, 100], f32, tag="xstage")
                nc.gpsimd.indirect_dma_start(
                    out=stg[:], out_offset=None, in_=tbl,
                    in_offset=bass.IndirectOffsetOnAxis(ap=idx[:, q:q + 1], axis=0))
                pt = psum.tile([100, 128], f32, tag="ps")
                nc.tensor.transpose(out=pt[:], in_=stg[:], identity=identf_t[:])
                nc.vector.tensor_copy(dst[0:100, q * 128:(q + 1) * 128], pt[:])

        # ---------- xg pre-GEMM ----------
        xg = [big.tile([128, NT * 800], bf16, tag=f"xg{d}", name=f"xg{d}") for d in (0, 1)]
        for d in (0, 1):
            for q in (range(NT) if d == 0 else reversed(range(NT))):
                for half in (0, 1):
                    pg = psum.tile([128, 400], f32, tag="ps")
                    nc.tensor.matmul(out=pg[:], lhsT=xT0[:, q * 128:(q + 1) * 128],
                                     rhs=wx_t[(d, 0)][:, half * 400:half * 400 + 400],
                                     start=True, stop=False)
                    nc.tensor.matmul(out=pg[:], lhsT=xT1[:, q * 128:(q + 1) * 128],
                                     rhs=wx_t[(d, 1)][:, half * 400:half * 400 + 400],
                                     start=False, stop=True)
                    nc.vector.tensor_copy(
                        xg[d][:, q * 800 + half * 400:q * 800 + half * 400 + 400], pg[:])

        # ---------- subword gathers (overlap with recurrence) ----------
        subp_all = big.tile([128, 8 * BW], f32, tag="subp")
        sube_all = big.tile([128, 8 * BW], f32, tag="sube")
        for i in range(8):
            for q in range(NQA):
                col = i * NQA + q
                for tbl, idx, dst in ((preP, spidx_t, subp_all), (embP, scidx_t, sube_all)):
                    nc.gpsimd.indirect_dma_start(
                        out=dst[:, i * BW + q * 4:i * BW + q * 4 + 4],
                        out_offset=None, in_=tbl,
                        in_offset=bass.IndirectOffsetOnAxis(ap=idx[:, col:col + 1], axis=0))

        # ---------- LSTM recurrence ----------
        hT = [big.tile([100, 2 * 4 * SP], bf16, tag=f"hT{d}", name=f"hT{d}") for d in (0, 1)]
        c_st = [big.tile([64, 100], f32, tag=f"c{d}", name=f"c{d}") for d in (0, 1)]
        for d in (0, 1):
            nc.vector.memset(c_st[d][:], 0.0)

        def emit_transpose(d, tx, h):
            # h [64, 100] bf16 -> hT[d] columns at time tx
            pt = psum.tile([100, 64], bf16, tag="ps", name="pt")
            nc.tensor.transpose(out=pt[:], in_=h[:], identity=identb_t[0:64, 0:64])
            src = pt[:].rearrange("p (e r) -> p e r", e=2)[:, :, 0:4]
            dst = hT[d][:].rearrange("p (e b t) -> p e b t", e=2, b=4)[:, :, :, tx]
            nc.vector.tensor_copy(dst, src)

        h_prev = [None, None]
        for t in range(S):
            # ---- B) per dir: transpose prev h, then PE block ----
            pg_d = []
            for d in (0, 1):
                if t > 0:
                    emit_transpose(d, t - 1 if d == 0 else S - t, h_prev[d])
                tx = t if d == 0 else S - 1 - t
                pg = psum.tile([64, 400], f32, tag="ps", name="pg")
                pg_d.append(pg)
                q, a, g8 = tx // 32, (tx % 32) // 8, (tx % 32) % 8
                for e in (0, 1):
                    nc.tensor.matmul(
                        out=pg[32 * e:32 * e + 32, :],
                        lhsT=isel_t[32 * a:32 * a + 32, g8 * 32:(g8 + 1) * 32],
                        rhs=xg[d][32 * a:32 * a + 32,
                                  q * 800 + 400 * e:q * 800 + 400 * e + 400],
                        start=True, stop=False,
                        tile_position=(32 * a, 32 * e))
                if t > 0:
                    tp = tx + 1 if d == 1 else tx - 1
                    hview = hT[d][:].rearrange("p (e b t) -> p e b t", e=2, b=4)
                    for e_dst in (0, 1):
                        for e_src in (0, 1):
                            nc.tensor.matmul(
                                out=pg[32 * e_dst:32 * e_dst + 4, :],
                                lhsT=hview[:, e_src, :, tp],
                                rhs=wh_t[(d, e_src)][:, 400 * e_dst:400 * e_dst + 400],
                                start=False, stop=False)
                for e in (0, 1):
                    # zero-adding N=1 matmul: marks the bank region stopped
                    nc.tensor.matmul(
                        out=pg[32 * e:32 * e + 32, 0:1],
                        lhsT=zsel_t[32 * a:32 * a + 32, :],
                        rhs=xg[d][32 * a:32 * a + 32, 0:1],
                        start=False, stop=True,
                        tile_position=(32 * a, 32 * e))
            # ---- C) gate activations (sigmoids first, both dirs) ----
            sg_d, tct_d = [], []
            for d in (0, 1):
                sg = step.tile([64, 400], f32, tag="sg", name="sg")
                sg_d.append(sg)
                nc.scalar.activation(sg[:, 0:300], pg_d[d][:, 0:300], AF.Sigmoid)
                nc.scalar.activation(sg[:, 300:400], pg_d[d][:, 300:400], AF.Tanh)
            # ---- D) DVE c-updates (both dirs), then tanh(c), then h ----
            for d in (0, 1):
                sg = sg_d[d]
                si, sf, tg = sg[:, 0:100], sg[:, 100:200], sg[:, 300:400]
                u = step.tile([64, 100], f32, tag="u", name="u")
                m1 = step.tile([64, 100], f32, tag="m1", name="m1")
                nc.vector.tensor_tensor(out=m1[:], in0=sf, in1=c_st[d][:], op=OP.mult)
                nc.vector.tensor_tensor(out=u[:], in0=si, in1=tg, op=OP.mult)
                nc.vector.tensor_tensor(out=c_st[d][:], in0=m1[:], in1=u[:], op=OP.add)
            for d in (0, 1):
                tct = step.tile([64, 100], f32, tag="tct", name="tct")
                tct_d.append(tct)
                nc.scalar.activation(tct[:], c_st[d][:], AF.Tanh)
            for d in (0, 1):
                h = step.tile([64, 100], bf16, tag="h", name="h")
                nc.vector.tensor_tensor(out=h[:], in0=sg_d[d][:, 200:300],
                                        in1=tct_d[d][:], op=OP.mult)
                h_prev[d] = h
        for d in (0, 1):
            emit_transpose(d, S - 1 if d == 0 else 0, h_prev[d])

        # ---------- P projections -> DRAM staging ----------
        for d in (0, 1):
            hview = hT[d][:].rearrange("p (e b t) -> p e b t", e=2, b=4)
            for b_ in range(4):
                for m in range(S // 128):
                    pp = psum.tile([128, 4], f32, tag="ps")
                    for e in (0, 1):
                        nc.tensor.matmul(
                            out=pp[:], lhsT=hview[:, e, b_, 128 * m:128 * m + 128],
                            rhs=wp_t[d][e][:], start=(e == 0), stop=(e == 1))
                    pst = work.tile([128, 4], f32, tag="pstage")
                    nc.vector.tensor_copy(pst[:], pp[:])
                    nc.sync.dma_start(p_dram[d][b_, 128 * m:128 * m + 128, :], pst[:])
                nc.sync.dma_start(p_dram[d][b_, S:SP, :], zeros_t[0:SP - S, :])

        # ---------- FFN assembly ----------
        def pload(dram, shift, tag):
            t = work.tile([128, BW], f32, tag=tag, name=tag)
            dstv = t[:].rearrange("p (b qq l) -> p b qq l", b=4, qq=nqb)
            for b_ in range(4):
                src = dram[b_, shift:shift + S, :].rearrange("(qq p) l -> p qq l", p=128)
                nc.sync.dma_start(dstv[:, b_, :, :], src)
            return t

        pfk = pload(p_dram[0], 0, "pfk")
        pbk1 = pload(p_dram[1], 1, "pbk1")
        n_full = SK // 128
        rem = SK % 128
        for i in range(8):
            pfe1 = pload(p_dram[0], i + 1, "pfe1")
            pbe2 = pload(p_dram[1], i + 2, "pbe2")
            d1 = work.tile([128, BW], f32, tag="d1")
            d2 = work.tile([128, BW], f32, tag="d2")
            lg = work.tile([128, BW], f32, tag="lg")
            nc.vector.tensor_tensor(out=d1[:], in0=pfe1[:], in1=pfk[:], op=OP.subtract)
            nc.vector.tensor_tensor(out=d2[:], in0=pbk1[:], in1=pbe2[:], op=OP.subtract)
            nc.vector.tensor_tensor(out=d1[:], in0=d1[:], in1=d2[:], op=OP.add)
            nc.vector.tensor_tensor(out=d1[:], in0=d1[:],
                                    in1=mask_t[:, i * BW:(i + 1) * BW], op=OP.mult)
            nc.vector.tensor_tensor(out=d1[:], in0=d1[:],
                                    in1=subp_all[:, i * BW:(i + 1) * BW], op=OP.add)
            nc.vector.tensor_tensor(out=lg[:], in0=d1[:],
                                    in1=sube_all[:, i * BW:(i + 1) * BW], op=OP.add)
            lgv = lg[:].rearrange("p (b qq l) -> p b qq l", b=4, qq=nqb)
            for b_ in range(4):
                if n_full > 0:
                    dstv = out_d[b_, 0:128 * n_full, i, :].rearrange(
                        "(qq p) l -> p qq l", p=128)
                    nc.sync.dma_start(dstv, lgv[:, b_, 0:n_full, :])
                if rem > 0:
                    dstv = out_d[b_, 128 * n_full:SK, i, :]
                    nc.sync.dma_start(dstv, lgv[0:rem, b_, n_full, :])

    nc.compile()
    return nc


def get_program(S):
    import kernel as _self  # noqa
    key = ("prog", S)
    if key not in _prog_cache:
        _prog_cache[key] = build_program(S)
    return _prog_cache[key]


_prog_cache = {}


def kernel(**inputs):
    S = int(np.asarray(inputs["chars"]).shape[1])
    nc = get_program(S)
    in_maps = prep_host(inputs, S)
    from concourse import bass_utils
    res = bass_utils.run_bass_kernel_spmd(nc, in_maps, list(range(NCORES)))
    outs = [res.results[c]["out"] for c in range(NCORES)]
    return np.concatenate(outs, axis=0).astype(np.float32)
